# revision 8
# baseline (speedup 1.0000x reference)
"""Trainium2 Bass kernel for additive attention (nn_Attention).

Reference computation (per batch b):
    att_h  = h @ W.T + b_h2att                      [B, ATTH]
    dot    = tanh(p_att_feats + att_h[:, None, :])  [B, S, ATTH]
    scores = dot @ w_alpha[0] (+ b_alpha)           [B, S]
    weight = softmax(scores, axis=1)
    out    = weight @ att_feats                     [B, RNN]

Sharding: data-parallel over batch, 32 batches per core x 8 cores.

Per-core layout: (batch, S) flattened to G = 32*196 = 6272 rows
= exactly 49 tiles of 128 partitions. Per tile t:
  - z = p_tile + att_h[row's batch] computed on TensorE in PSUM
    (identity matmul streams p, then a 0/1 mask matmul adds the
    correct batch's att_h row; the h2att bias is folded into att_h)
  - dot = tanh(z) on ScalarE
  - scores col = sum_a dot * w_alpha via fused tensor_tensor_reduce (VectorE)
  - e = exp(scores) unnormalized (softmax shift bounded: |scores| <~ 20,
    b_alpha cancels in softmax so it is dropped entirely)
  - masked weight columns lhsT[p, b] = e[p] * (batch(p)==b)
  - att_res += lhsT.T @ att_tile on TensorE (per-batch rows in PSUM)
  - sumexp  += lhsT.T @ ones
Final: out = att_res * (1/sumexp) fused into the PSUM->SBUF copy.
"""

import numpy as np

import concourse.bass as bass
import concourse.tile as tile
from concourse import bacc, mybir
from concourse.bass_utils import run_bass_kernel_spmd

F32 = mybir.dt.float32
F32R = mybir.dt.float32r
AF = mybir.ActivationFunctionType
ALU = mybir.AluOpType

B, S, RNN, ATTH = 256, 196, 1024, 512
NCORES = 8
BSH = B // NCORES            # 32 batches per core
G = BSH * S                  # 6272 rows per core
NT = G // 128                # 49 tiles
assert NT * 128 == G

P_GROUP = 7                  # p_att tiles per DMA (7 * 256KB = 1.75MB)
A_GROUP = 7                  # att_feats tiles per DMA (7 * 512KB = 3.5MB)

_cached = {}


def _batch_of_row(g):
    return g // S


def build_nc():
    nc = bacc.Bacc("TRN2", target_bir_lowering=False, debug=False,
                   enable_asserts=True, num_devices=NCORES)

    h_d = nc.dram_tensor("h", [BSH, RNN], F32, kind="ExternalInput")
    att_d = nc.dram_tensor("att", [G, RNN], F32, kind="ExternalInput")
    p_d = nc.dram_tensor("p_att", [G, ATTH], F32, kind="ExternalInput")
    w_d = nc.dram_tensor("w_h2att", [ATTH, RNN], F32, kind="ExternalInput")
    bias_d = nc.dram_tensor("b_h2att", [1, ATTH], F32, kind="ExternalInput")
    walpha_d = nc.dram_tensor("w_alpha", [1, ATTH], F32, kind="ExternalInput")
    out_d = nc.dram_tensor("out", [BSH, RNN], F32, kind="ExternalOutput")

    # --- host-side constants, embedded in the NEFF ---
    ident_np = np.eye(128, dtype=np.float32)
    ones_np = np.ones((128, 128), dtype=np.float32)
    # maskT[p, t, b] = 1 if batch(128t + p) == b
    maskT_np = np.zeros((128, NT, BSH), dtype=np.float32)
    for t in range(NT):
        for p in range(128):
            bb = _batch_of_row(128 * t + p)
            maskT_np[p, t, bb] = 1.0
    # bsel[b, t, p]: one-hot selector, bsel.T @ att_h broadcasts per-row att_h
    bsel_np = np.ascontiguousarray(maskT_np.transpose(2, 1, 0))

    ident_c = nc.inline_tensor(ident_np, "c_ident")
    ones_c = nc.inline_tensor(ones_np, "c_ones")
    bsel_c = nc.inline_tensor(bsel_np.reshape(BSH, NT * 128), "c_bsel")
    maskT_c = nc.inline_tensor(maskT_np.reshape(128, NT * BSH), "c_maskT")

    with tile.TileContext(nc) as tc:
        import contextlib
        ctx = contextlib.ExitStack()
        with ctx:
            consts = ctx.enter_context(tc.tile_pool(name="consts", bufs=1))
            work = ctx.enter_context(tc.tile_pool(name="work", bufs=1))
            p_pool = ctx.enter_context(tc.tile_pool(name="p_pool", bufs=2))
            a_pool = ctx.enter_context(tc.tile_pool(name="a_pool", bufs=2))
            dot_pool = ctx.enter_context(tc.tile_pool(name="dot", bufs=3))
            prod_pool = ctx.enter_context(tc.tile_pool(name="prod", bufs=2))
            small_pool = ctx.enter_context(tc.tile_pool(name="small", bufs=4))
            zp_pool = ctx.enter_context(
                tc.tile_pool(name="zpsum", bufs=3, space="PSUM"))
            res_pool = ctx.enter_context(
                tc.tile_pool(name="respsum", bufs=1, space="PSUM"))
            setup_ps = ctx.enter_context(
                tc.tile_pool(name="setupps", bufs=2, space="PSUM"))

            # ---- load constants and small inputs ----
            ident_sb = consts.tile([128, 128], F32R)
            nc.sync.dma_start(out=ident_sb[:], in_=ident_c[:].bitcast(F32R))
            ident32_sb = consts.tile([128, 128], F32)
            nc.sync.dma_start(out=ident32_sb[:], in_=ident_c[:])
            ones_sb = consts.tile([128, 128], F32)
            nc.sync.dma_start(out=ones_sb[:], in_=ones_c[:])
            onesr_sb = consts.tile([128, 2], F32R)
            nc.sync.dma_start(out=onesr_sb[:], in_=ones_c[:, 0:2].bitcast(F32R))
            bsel_sb = consts.tile([BSH, NT * 128], F32R)
            nc.sync.dma_start(out=bsel_sb[:], in_=bsel_c[:].bitcast(F32R))
            maskT_sb = consts.tile([128, NT * BSH], F32)
            nc.sync.dma_start(out=maskT_sb[:], in_=maskT_c[:])

            h_sb = work.tile([BSH, RNN], F32)
            nc.sync.dma_start(out=h_sb[:], in_=h_d[:])
            w_sb = work.tile([128, 4 * RNN], F32)
            nc.sync.dma_start(
                out=w_sb[:].rearrange("p (c r) -> p c r", c=4),
                in_=w_d[:].rearrange("(c p) r -> p c r", p=128))
            bias_sb = work.tile([1, ATTH], F32)
            nc.sync.dma_start(out=bias_sb[:], in_=bias_d[:])
            walpha_sb = work.tile([1, ATTH], F32)
            nc.sync.dma_start(out=walpha_sb[:], in_=walpha_d[:])

            # ---- transpose h -> hT [r, b] and W -> wT [r, a] via PE ----
            hT_sb = work.tile([128, 8 * BSH], F32)
            for rc in range(8):
                ps = setup_ps.tile([128, BSH], F32, tag="sps")
                nc.tensor.transpose(
                    ps[:], h_sb[:, rc * 128:(rc + 1) * 128],
                    ident32_sb[0:BSH, 0:BSH])
                nc.vector.tensor_copy(hT_sb[:, rc * BSH:(rc + 1) * BSH], ps[:])
            wT_sb = work.tile([128, 8 * ATTH], F32)
            for rc in range(8):
                for ac in range(4):
                    ps = setup_ps.tile([128, 128], F32, tag="sps")
                    nc.tensor.transpose(
                        ps[:], w_sb[:, ac * RNN + rc * 128: ac * RNN + (rc + 1) * 128],
                        ident32_sb[:, :])
                    nc.vector.tensor_copy(
                        wT_sb[:, rc * ATTH + ac * 128: rc * ATTH + (ac + 1) * 128],
                        ps[:])

            # ---- att_h = h @ W.T + bias  -> [BSH, ATTH] (fp32 exact) ----
            ah_ps = setup_ps.tile([BSH, ATTH], F32, tag="sps")
            for rc in range(8):
                nc.tensor.matmul(
                    ah_ps[:],
                    lhsT=hT_sb[:, rc * BSH:(rc + 1) * BSH],
                    rhs=wT_sb[:, rc * ATTH:(rc + 1) * ATTH],
                    start=(rc == 0), stop=False)
            nc.tensor.matmul(
                ah_ps[:], lhsT=ones_sb[0:1, 0:BSH], rhs=bias_sb[0:1, :],
                start=False, stop=True)
            atth_sb = work.tile([BSH, ATTH], F32R)
            nc.vector.tensor_copy(atth_sb[:], ah_ps[:])

            # ---- broadcast w_alpha to all 128 partitions ----
            wb_ps = setup_ps.tile([128, ATTH], F32, tag="sps")
            nc.tensor.matmul(wb_ps[:], lhsT=ones_sb[0:1, 0:128],
                             rhs=walpha_sb[0:1, :], start=True, stop=True)
            wb_sb = work.tile([128, ATTH], F32)
            nc.vector.tensor_copy(wb_sb[:], wb_ps[:])

            # ---- persistent accumulators ----
            res_ps0 = res_pool.tile([BSH, 512], F32, tag="res0")
            res_ps1 = res_pool.tile([BSH, 512], F32, tag="res1")
            se_ps = res_pool.tile([BSH, 2], F32, tag="sumexp")

            p_view = p_d[:].rearrange("(t p) e -> p t e", p=128)
            a_view = att_d[:].rearrange("(t p) e -> p t e", p=128)

            n_pg = (NT + P_GROUP - 1) // P_GROUP
            n_ag = (NT + A_GROUP - 1) // A_GROUP
            p_tiles = {}
            a_tiles = {}

            def load_p_group(g):
                lo = g * P_GROUP
                hi = min(NT, lo + P_GROUP)
                t_ = p_pool.tile([128, (hi - lo) * ATTH], F32R, tag="pg")
                nc.sync.dma_start(
                    out=t_[:].rearrange("p (t e) -> p t e", e=ATTH),
                    in_=p_view[:, lo:hi, :].bitcast(F32R))
                for t in range(lo, hi):
                    p_tiles[t] = t_[:, (t - lo) * ATTH:(t - lo + 1) * ATTH]

            def load_a_group(g):
                lo = g * A_GROUP
                hi = min(NT, lo + A_GROUP)
                t_ = a_pool.tile([128, (hi - lo) * RNN], F32R, tag="ag")
                nc.sync.dma_start(
                    out=t_[:].rearrange("p (t e) -> p t e", e=RNN),
                    in_=a_view[:, lo:hi, :].bitcast(F32R))
                for t in range(lo, hi):
                    a_tiles[t] = t_[:, (t - lo) * RNN:(t - lo + 1) * RNN]

            # ---- main loop over 49 row-tiles, grouped by DMA group ----
            assert P_GROUP == A_GROUP
            n_groups = (NT + P_GROUP - 1) // P_GROUP
            for g in range(n_groups):
                lo = g * P_GROUP
                hi = min(NT, lo + P_GROUP)
                load_p_group(g)
                load_a_group(g)

                scol_g = small_pool.tile([128, hi - lo], F32, tag="scol")
                for t in range(lo, hi):
                    # z = p + att_h[batch] in PSUM
                    z_ps = zp_pool.tile([128, ATTH], F32, tag="z")
                    nc.tensor.matmul(
                        z_ps[:], lhsT=ident_sb[:],
                        rhs=p_tiles[t], start=True, stop=False)
                    nc.tensor.matmul(
                        z_ps[:],
                        lhsT=bsel_sb[:, t * 128:(t + 1) * 128],
                        rhs=atth_sb[:],
                        start=False, stop=True)

                    # dot = tanh(z) -> SBUF
                    dot_sb = dot_pool.tile([128, ATTH], F32, tag="dot")
                    nc.scalar.activation(dot_sb[:], z_ps[:], AF.Tanh)

                    # prod = dot * w_alpha (VectorE)
                    prod_sb = prod_pool.tile([128, ATTH], F32, tag="prod")
                    nc.vector.tensor_tensor(
                        out=prod_sb[:], in0=dot_sb[:], in1=wb_sb[:],
                        op=ALU.mult)
                    # scores col = sum_a prod (ScalarE accumulate)
                    junk_sb = prod_pool.tile([128, ATTH], F32, tag="junk")
                    nc.scalar.activation(
                        junk_sb[:], prod_sb[:], AF.Copy, bias=0.0, scale=1.0,
                        accum_out=scol_g[:, t - lo: t - lo + 1])

                # e = exp(scores) for the whole group
                ecol_g = small_pool.tile([128, hi - lo], F32, tag="ecol")
                nc.scalar.activation(ecol_g[:], scol_g[:], AF.Exp)

                for t in range(lo, hi):
                    # masked weight columns: lhsT[p, b] = e[p] * mask[p, b]
                    lhsT_t = small_pool.tile([128, BSH], F32R, tag="lhsT")
                    nc.vector.tensor_scalar(
                        out=lhsT_t[:], in0=maskT_sb[:, t * BSH:(t + 1) * BSH],
                        scalar1=ecol_g[:, t - lo: t - lo + 1], scalar2=None,
                        op0=ALU.mult)

                    # att_res += lhsT.T @ A ; sumexp += lhsT.T @ 1
                    nc.tensor.matmul(
                        res_ps0[:], lhsT=lhsT_t[:],
                        rhs=a_tiles[t][:, 0:512],
                        start=(t == 0), stop=(t == NT - 1))
                    nc.tensor.matmul(
                        res_ps1[:], lhsT=lhsT_t[:],
                        rhs=a_tiles[t][:, 512:1024],
                        start=(t == 0), stop=(t == NT - 1))
                    nc.tensor.matmul(
                        se_ps[:], lhsT=lhsT_t[:], rhs=onesr_sb[:],
                        start=(t == 0), stop=(t == NT - 1))

            # ---- finalize: out = att_res / sumexp ----
            recip_sb = work.tile([BSH, 1], F32)
            nc.vector.reciprocal(recip_sb[:], se_ps[:, 0:1])
            out_sb = work.tile([BSH, RNN], F32)
            nc.scalar.activation(out_sb[:, 0:512], res_ps0[:], AF.Copy,
                                 bias=0.0, scale=recip_sb[:, 0:1])
            nc.scalar.activation(out_sb[:, 512:1024], res_ps1[:], AF.Copy,
                                 bias=0.0, scale=recip_sb[:, 0:1])
            nc.sync.dma_start(out=out_d[:], in_=out_sb[:])

    nc.compile()
    return nc


def kernel(h, att_feats, p_att_feats, w_h2att, b_h2att, w_alpha, b_alpha):
    """Full-input entry point. b_alpha is dropped: softmax is shift-invariant."""
    if "nc" not in _cached:
        _cached["nc"] = build_nc()
    nc = _cached["nc"]

    h = np.asarray(h, dtype=np.float32)
    att_feats = np.asarray(att_feats, dtype=np.float32)
    p_att_feats = np.asarray(p_att_feats, dtype=np.float32)
    w_h2att = np.ascontiguousarray(np.asarray(w_h2att, dtype=np.float32))
    b_h2att = np.asarray(b_h2att, dtype=np.float32).reshape(1, ATTH)
    w_alpha = np.asarray(w_alpha, dtype=np.float32).reshape(1, ATTH)

    in_maps = []
    for c in range(NCORES):
        lo = c * BSH
        hi = lo + BSH
        in_maps.append({
            "h": np.ascontiguousarray(h[lo:hi]),
            "att": np.ascontiguousarray(
                att_feats[lo:hi].reshape(G, RNN)),
            "p_att": np.ascontiguousarray(
                p_att_feats[lo:hi].reshape(G, ATTH)),
            "w_h2att": w_h2att,
            "b_h2att": b_h2att,
            "w_alpha": w_alpha,
        })

    res = run_bass_kernel_spmd(nc, in_maps, list(range(NCORES)))
    out = np.concatenate([res.results[c]["out"] for c in range(NCORES)],
                         axis=0)
    return out.astype(np.float32)


# revision 23
# speedup vs baseline: 116.9162x; 116.9162x over previous
"""Trainium2 Bass kernel for additive attention (nn_Attention).

Reference computation (per batch b):
    att_h  = h @ W.T + b_h2att                      [B, ATTH]
    dot    = tanh(p_att_feats + att_h[:, None, :])  [B, S, ATTH]
    scores = dot @ w_alpha[0] (+ b_alpha)           [B, S]
    weight = softmax(scores, axis=1)
    out    = weight @ att_feats                     [B, RNN]

Sharding: data-parallel over batch, 32 batches per core x 8 cores.

Per-core layout: (batch, S) flattened to G = 32*196 = 6272 rows
= exactly 49 tiles of 128 partitions. Per tile t:
  - z = p_tile + att_h[row's batch] computed on TensorE in PSUM
    (identity matmul streams p, then a 0/1 mask matmul adds the
    correct batch's att_h row; the h2att bias is folded into att_h)
  - dot = tanh(z) on ScalarE
  - scores col = sum_a dot * w_alpha via fused tensor_tensor_reduce (VectorE)
  - e = exp(scores) unnormalized (softmax shift bounded: |scores| <~ 20,
    b_alpha cancels in softmax so it is dropped entirely)
  - masked weight columns lhsT[p, b] = e[p] * (batch(p)==b)
  - att_res += lhsT.T @ att_tile on TensorE (per-batch rows in PSUM)
  - sumexp  += lhsT.T @ ones
Final: out = att_res * (1/sumexp) fused into the PSUM->SBUF copy.
"""

import numpy as np

import concourse.bass as bass
import concourse.tile as tile
from concourse import bacc, mybir
from concourse.bass_utils import run_bass_kernel_spmd

F32 = mybir.dt.float32
F32R = mybir.dt.float32r
AF = mybir.ActivationFunctionType
ALU = mybir.AluOpType

B, S, RNN, ATTH = 256, 196, 1024, 512
NCORES = 8
BSH = B // NCORES            # 32 batches per core
G = BSH * S                  # 6272 rows per core
NT = G // 128                # 49 tiles
assert NT * 128 == G
GROUPS = [7, 7, 7, 7, 7, 7, 7]  # tiles per DMA group
assert sum(GROUPS) == NT
GSTART = [sum(GROUPS[:i]) for i in range(len(GROUPS))]

_cached = {}


def _batch_of_row(g):
    return g // S


def _rep_groups(repeats):
    for r in range(repeats):
        for g in range(len(GROUPS)):
            yield r, g


def build_nc(repeats=1):
    nc = bacc.Bacc("TRN2", target_bir_lowering=False, debug=False,
                   enable_asserts=True, num_devices=NCORES)

    h_d = nc.dram_tensor("h", [BSH, RNN], F32, kind="ExternalInput")
    att_d = nc.dram_tensor("att", [G, RNN], F32, kind="ExternalInput")
    p_d = nc.dram_tensor("p_att", [G, ATTH], F32, kind="ExternalInput")
    w_d = nc.dram_tensor("w_h2att", [ATTH, RNN], F32, kind="ExternalInput")
    bias_d = nc.dram_tensor("b_h2att", [1, ATTH], F32, kind="ExternalInput")
    walpha_d = nc.dram_tensor("w_alpha", [1, ATTH], F32, kind="ExternalInput")
    out_d = nc.dram_tensor("out", [BSH, RNN], F32, kind="ExternalOutput")

    # --- host-side constants, embedded in the NEFF ---
    ident_np = np.eye(128, dtype=np.float32)
    ones_np = np.ones((128, 128), dtype=np.float32)
    # maskT[p, t, b] = 1 if batch(128t + p) == b
    maskT_np = np.zeros((128, NT, BSH), dtype=np.float32)
    for t in range(NT):
        for p in range(128):
            bb = _batch_of_row(128 * t + p)
            maskT_np[p, t, bb] = 1.0
    # bsel[b, t, p]: one-hot selector, bsel.T @ att_h broadcasts per-row att_h
    bsel_np = np.ascontiguousarray(maskT_np.transpose(2, 1, 0))

    ident_c = nc.inline_tensor(ident_np, "c_ident")
    ones_c = nc.inline_tensor(ones_np, "c_ones")
    bsel_c = nc.inline_tensor(bsel_np.reshape(BSH, NT * 128), "c_bsel")
    maskT_c = nc.inline_tensor(maskT_np.reshape(128, NT * BSH), "c_maskT")

    with tile.TileContext(nc) as tc:
        import contextlib
        ctx = contextlib.ExitStack()
        with ctx:
            consts = ctx.enter_context(tc.tile_pool(name="consts", bufs=1))
            work = ctx.enter_context(tc.tile_pool(name="work", bufs=1))
            p_pool = ctx.enter_context(tc.tile_pool(name="p_pool", bufs=2))
            a_pool = ctx.enter_context(tc.tile_pool(name="a_pool", bufs=3))
            setup_sb_cm = tc.tile_pool(name="setup_sb", bufs=1)
            setup_sb = setup_sb_cm.__enter__()
            res_pool = ctx.enter_context(
                tc.tile_pool(name="respsum", bufs=1, space="PSUM"))
            setup_ps_cm = tc.tile_pool(name="setupps", bufs=2, space="PSUM")
            setup_ps = setup_ps_cm.__enter__()

            # ---- load constants and small inputs ----
            # W first on the ACT ring: it gates the att_h chain
            w_sb = setup_sb.tile([128, 4 * RNN], F32)
            w_view = w_d[:].rearrange("(c p) r -> p c r", p=128)
            for wc in range(4):
                nc.scalar.dma_start(
                    out=w_sb[:, wc * RNN:(wc + 1) * RNN],
                    in_=w_view[:, wc, :])
            h_sb = setup_sb.tile([BSH, RNN], F32)
            nc.scalar.dma_start(out=h_sb[:], in_=h_d[:])
            ident_sb = consts.tile([128, 128], F32R)
            nc.scalar.dma_start(out=ident_sb[:], in_=ident_c[:].bitcast(F32R))
            ident32_sb = consts.tile([128, 128], F32)
            nc.scalar.dma_start(out=ident32_sb[:], in_=ident_c[:])
            ones_sb = consts.tile([128, 128], F32R)
            nc.scalar.dma_start(out=ones_sb[:], in_=ones_c[:].bitcast(F32R))
            onesr_sb = consts.tile([128, 2], F32R)
            nc.scalar.dma_start(out=onesr_sb[:], in_=ones_c[:, 0:2].bitcast(F32R))
            bsel_sb = consts.tile([BSH, NT * 128], F32R)
            nc.scalar.dma_start(out=bsel_sb[:], in_=bsel_c[:].bitcast(F32R))
            bias_sb = setup_sb.tile([1, ATTH], F32)
            nc.scalar.dma_start(out=bias_sb[:], in_=bias_d[:])
            ones32_sb = consts.tile([1, 128], F32)
            nc.scalar.dma_start(out=ones32_sb[:], in_=ones_c[0:1, :])
            walpha_sb = setup_sb.tile([1, ATTH], F32R)
            nc.scalar.dma_start(out=walpha_sb[:], in_=walpha_d[:].bitcast(F32R))
            maskT_sb = consts.tile([128, NT * BSH], F32)
            nc.scalar.dma_start(out=maskT_sb[:], in_=maskT_c[:])

            # ---- transpose h -> hT [r, b] and W -> wT [r, a] via PE ----
            hT_sb = setup_sb.tile([128, 8 * BSH], F32)
            for rc in range(8):
                ps = setup_ps.tile([128, BSH], F32, tag="sps")
                nc.tensor.transpose(
                    ps[:], h_sb[:, rc * 128:(rc + 1) * 128],
                    ident32_sb[0:BSH, 0:BSH])
                nc.vector.tensor_copy(hT_sb[:, rc * BSH:(rc + 1) * BSH], ps[:])
            wT_sb = setup_sb.tile([128, 8 * ATTH], F32)
            for rc in range(8):
                for ac in range(4):
                    ps = setup_ps.tile([128, 128], F32, tag="sps")
                    nc.tensor.transpose(
                        ps[:], w_sb[:, ac * RNN + rc * 128: ac * RNN + (rc + 1) * 128],
                        ident32_sb[:, :])
                    nc.vector.tensor_copy(
                        wT_sb[:, rc * ATTH + ac * 128: rc * ATTH + (ac + 1) * 128],
                        ps[:])

            # ---- att_h = h @ W.T + bias  -> [BSH, ATTH] (fp32 exact) ----
            ah_ps = setup_ps.tile([BSH, ATTH], F32, tag="sps")
            for rc in range(8):
                nc.tensor.matmul(
                    ah_ps[:],
                    lhsT=hT_sb[:, rc * BSH:(rc + 1) * BSH],
                    rhs=wT_sb[:, rc * ATTH:(rc + 1) * ATTH],
                    start=(rc == 0), stop=False)
            nc.tensor.matmul(
                ah_ps[:], lhsT=ones32_sb[0:1, 0:BSH], rhs=bias_sb[0:1, :],
                start=False, stop=True)
            atth32_sb = work.tile([BSH, ATTH], F32)
            nc.vector.tensor_copy(atth32_sb[:], ah_ps[:])
            atth_sb = work.tile([BSH, ATTH], F32R)
            nc.scalar.activation(atth_sb[:], atth32_sb[:], AF.Copy,
                                 bias=0.0, scale=1.0)

            # ---- broadcast w_alpha to all 128 partitions ----
            wb_ps = setup_ps.tile([128, ATTH], F32, tag="sps")
            nc.tensor.matmul(wb_ps[:], lhsT=ones_sb[0:1, 0:128],
                             rhs=walpha_sb[0:1, :], start=True, stop=True)
            wb_sb = work.tile([128, ATTH], F32)
            nc.vector.tensor_copy(wb_sb[:], wb_ps[:])
            setup_sb_cm.__exit__(None, None, None)
            setup_ps_cm.__exit__(None, None, None)

            zp_pool = ctx.enter_context(
                tc.tile_pool(name="zpsum", bufs=2, space="PSUM"))
            dot_pool = ctx.enter_context(tc.tile_pool(name="dot", bufs=3))
            prod_pool = ctx.enter_context(tc.tile_pool(name="prod", bufs=2))
            small_pool = ctx.enter_context(tc.tile_pool(name="small", bufs=4))

            # ---- persistent accumulators ----
            res_ps0 = res_pool.tile([BSH, 512], F32, tag="res0")
            res_ps1 = res_pool.tile([BSH, 512], F32, tag="res1")
            se_ps = res_pool.tile([BSH, 2], F32, tag="sumexp")

            p_view = p_d[:].rearrange("(t p) e -> p t e", p=128)
            a_view = att_d[:].rearrange("(t p) e -> p t e", p=128)

            p_tiles = {}
            a_tiles = {}
            GMAX = max(GROUPS)

            def load_p_group(g):
                lo = GSTART[g]
                hi = lo + GROUPS[g]
                t_ = p_pool.tile([128, GMAX * ATTH], F32R, tag="pg")
                nc.gpsimd.dma_start(
                    out=t_[:, 0:(hi - lo) * ATTH].rearrange(
                        "p (t e) -> p t e", e=ATTH),
                    in_=p_view[:, lo:hi, :].bitcast(F32R))
                for t in range(lo, hi):
                    p_tiles[t] = t_[:, (t - lo) * ATTH:(t - lo + 1) * ATTH]

            def load_a_group(g):
                lo = GSTART[g]
                hi = lo + GROUPS[g]
                t_ = a_pool.tile([128, GMAX * RNN], F32R, tag="ag")
                nc.sync.dma_start(
                    out=t_[:, 0:(hi - lo) * RNN].rearrange(
                        "p (t e) -> p t e", e=RNN),
                    in_=a_view[:, lo:hi, :].bitcast(F32R))
                for t in range(lo, hi):
                    a_tiles[t] = t_[:, (t - lo) * RNN:(t - lo + 1) * RNN]

            # ---- main loop over 49 row-tiles, grouped by DMA group ----
            for _rep, g in _rep_groups(repeats):
                lo = GSTART[g]
                hi = lo + GROUPS[g]
                load_p_group(g)
                load_a_group(g)

                scol_g = small_pool.tile([128, GROUPS[g]], F32, tag="scol")
                # pair tiles: one [128, 1024] PSUM (2 banks), one tanh call
                pairs = [(t, min(t + 1, hi - 1)) if t + 1 < hi else (t, None)
                         for t in range(lo, hi, 2)]
                for t0, t1 in pairs:
                    n_z = 1024 if t1 is not None else 512
                    z_ps = zp_pool.tile([128, 1024], F32, tag="z")
                    for i, t in enumerate((t0, t1)):
                        if t is None:
                            continue
                        zslice = z_ps[:, i * 512:(i + 1) * 512]
                        nc.tensor.matmul(
                            zslice, lhsT=ident_sb[:],
                            rhs=p_tiles[t], start=True, stop=False)
                        nc.tensor.matmul(
                            zslice,
                            lhsT=bsel_sb[:, t * 128:(t + 1) * 128],
                            rhs=atth_sb[:],
                            start=False, stop=True)

                    # dot = tanh(z) -> SBUF (one call per pair)
                    dot_sb = dot_pool.tile([128, 1024], F32, tag="dot")
                    nc.scalar.activation(dot_sb[:, 0:n_z], z_ps[:, 0:n_z],
                                         AF.Tanh)

                    for i, t in enumerate((t0, t1)):
                        if t is None:
                            continue
                        dslice = dot_sb[:, i * 512:(i + 1) * 512]
                        # prod = dot * w_alpha (VectorE)
                        prod_sb = prod_pool.tile([128, ATTH], F32, tag="prod")
                        nc.vector.tensor_tensor(
                            out=prod_sb[:], in0=dslice, in1=wb_sb[:],
                            op=ALU.mult)
                        # scores col = sum_a prod; alternate engines to
                        # balance ScalarE vs VectorE occupancy
                        if t % 2 == 0:
                            nc.scalar.activation(
                                dslice, prod_sb[:], AF.Copy, bias=0.0,
                                scale=1.0,
                                accum_out=scol_g[:, t - lo: t - lo + 1])
                        else:
                            nc.vector.tensor_reduce(
                                out=scol_g[:, t - lo: t - lo + 1],
                                in_=prod_sb[:], axis=mybir.AxisListType.X,
                                op=ALU.add)

                # e = exp(scores) for the whole group
                ecol_g = small_pool.tile([128, GROUPS[g]], F32, tag="ecol")
                nc.scalar.activation(ecol_g[:], scol_g[:], AF.Exp)

                for t in range(lo, hi):
                    # masked weight columns: lhsT[p, b] = e[p] * mask[p, b]
                    lhsT_t = small_pool.tile([128, BSH], F32R, tag="lhsT")
                    nc.vector.tensor_scalar(
                        out=lhsT_t[:], in0=maskT_sb[:, t * BSH:(t + 1) * BSH],
                        scalar1=ecol_g[:, t - lo: t - lo + 1], scalar2=None,
                        op0=ALU.mult)

                    # att_res += lhsT.T @ A ; sumexp += lhsT.T @ 1
                    nc.tensor.matmul(
                        res_ps0[:], lhsT=lhsT_t[:],
                        rhs=a_tiles[t][:, 0:512],
                        start=(t == 0), stop=(t == NT - 1))
                    nc.tensor.matmul(
                        res_ps1[:], lhsT=lhsT_t[:],
                        rhs=a_tiles[t][:, 512:1024],
                        start=(t == 0), stop=(t == NT - 1))
                    nc.tensor.matmul(
                        se_ps[:], lhsT=lhsT_t[:], rhs=onesr_sb[:],
                        start=(t == 0), stop=(t == NT - 1))

                # ---- finalize: out = att_res / sumexp (per repeat so no
                # repeat is dead code in benchmark builds) ----
                if g == len(GROUPS) - 1:
                    recip_sb = work.tile([BSH, 1], F32)
                    nc.vector.reciprocal(recip_sb[:], se_ps[:, 0:1])
                    out_sb = work.tile([BSH, RNN], F32)
                    nc.scalar.activation(out_sb[:, 0:512], res_ps0[:], AF.Copy,
                                         bias=0.0, scale=recip_sb[:, 0:1])
                    nc.sync.dma_start(out=out_d[:, 0:512],
                                      in_=out_sb[:, 0:512])
                    nc.scalar.activation(out_sb[:, 512:1024], res_ps1[:],
                                         AF.Copy,
                                         bias=0.0, scale=recip_sb[:, 0:1])
                    nc.sync.dma_start(out=out_d[:, 512:1024],
                                      in_=out_sb[:, 512:1024])

    nc.compile()
    return nc


def kernel(h, att_feats, p_att_feats, w_h2att, b_h2att, w_alpha, b_alpha):
    """Full-input entry point. b_alpha is dropped: softmax is shift-invariant."""
    if "nc" not in _cached:
        _cached["nc"] = build_nc()
    nc = _cached["nc"]

    h = np.asarray(h, dtype=np.float32)
    att_feats = np.asarray(att_feats, dtype=np.float32)
    p_att_feats = np.asarray(p_att_feats, dtype=np.float32)
    w_h2att = np.ascontiguousarray(np.asarray(w_h2att, dtype=np.float32))
    b_h2att = np.asarray(b_h2att, dtype=np.float32).reshape(1, ATTH)
    w_alpha = np.asarray(w_alpha, dtype=np.float32).reshape(1, ATTH)

    in_maps = []
    for c in range(NCORES):
        lo = c * BSH
        hi = lo + BSH
        in_maps.append({
            "h": np.ascontiguousarray(h[lo:hi]),
            "att": np.ascontiguousarray(
                att_feats[lo:hi].reshape(G, RNN)),
            "p_att": np.ascontiguousarray(
                p_att_feats[lo:hi].reshape(G, ATTH)),
            "w_h2att": w_h2att,
            "b_h2att": b_h2att,
            "w_alpha": w_alpha,
        })

    res = run_bass_kernel_spmd(nc, in_maps, list(range(NCORES)))
    out = np.concatenate([res.results[c]["out"] for c in range(NCORES)],
                         axis=0)
    return out.astype(np.float32)


# revision 30
# speedup vs baseline: 231.6775x; 1.9816x over previous
"""Trainium2 Bass kernel for additive attention (nn_Attention).

Reference computation (per batch b):
    att_h  = h @ W.T + b_h2att                      [B, ATTH]
    dot    = tanh(p_att_feats + att_h[:, None, :])  [B, S, ATTH]
    scores = dot @ w_alpha[0] (+ b_alpha)           [B, S]
    weight = softmax(scores, axis=1)
    out    = weight @ att_feats                     [B, RNN]

Sharding: data-parallel over batch, 32 batches per core x 8 cores.

Per-core layout: (batch, S) flattened to G = 32*196 = 6272 rows
= exactly 49 tiles of 128 partitions. Per tile t:
  - z = p_tile + att_h[row's batch] computed on TensorE in PSUM
    (identity matmul streams p, then a 0/1 mask matmul adds the
    correct batch's att_h row; the h2att bias is folded into att_h)
  - dot = tanh(z) on ScalarE
  - scores col = sum_a dot * w_alpha: VectorE multiply, then the sum via
    ScalarE activation accum_out / VectorE tensor_reduce (alternating)
  - e = exp(scores) unnormalized (softmax shift bounded: |scores| <~ 20,
    b_alpha cancels in softmax so it is dropped entirely)
  - masked weight columns lhsT[p, b] = e[p] * (batch(p)==b)
  - att_res += lhsT.T @ att_tile on TensorE (per-batch rows in PSUM)
  - sumexp  += lhsT.T @ ones
Final: out = att_res * (1/sumexp) fused into the PSUM->SBUF copy.
"""

import numpy as np

import concourse.bass as bass
import concourse.tile as tile
from concourse import bacc, mybir
from concourse.bass_utils import run_bass_kernel_spmd

F32 = mybir.dt.float32
F32R = mybir.dt.float32r
AF = mybir.ActivationFunctionType
ALU = mybir.AluOpType

B, S, RNN, ATTH = 256, 196, 1024, 512
NCORES = 8
BSH = B // NCORES            # 32 batches per core
G = BSH * S                  # 6272 rows per core
NT = G // 128                # 49 tiles
assert NT * 128 == G
GROUPS = [4, 6, 7, 7, 7, 7, 7, 4]  # tiles per DMA group
assert sum(GROUPS) == NT
GSTART = [sum(GROUPS[:i]) for i in range(len(GROUPS))]

_cached = {}


def _batch_of_row(g):
    return g // S


def _rep_groups(repeats):
    for r in range(repeats):
        for g in range(len(GROUPS)):
            yield r, g


def build_nc(repeats=1):
    nc = bacc.Bacc("TRN2", target_bir_lowering=False, debug=False,
                   enable_asserts=True, num_devices=NCORES)

    h_d = nc.dram_tensor("h", [BSH, RNN], F32, kind="ExternalInput")
    att_d = nc.dram_tensor("att", [G, RNN], F32, kind="ExternalInput")
    p_d = nc.dram_tensor("p_att", [G, ATTH], F32, kind="ExternalInput")
    w_d = nc.dram_tensor("w_h2att", [ATTH, RNN], F32, kind="ExternalInput")
    bias_d = nc.dram_tensor("b_h2att", [1, ATTH], F32, kind="ExternalInput")
    walpha_d = nc.dram_tensor("w_alpha", [1, ATTH], F32, kind="ExternalInput")
    out_d = nc.dram_tensor("out", [BSH, RNN], F32, kind="ExternalOutput")

    # --- host-side constants, embedded in the NEFF ---
    ident_np = np.eye(128, dtype=np.float32)
    ones_np = np.ones((128, 128), dtype=np.float32)
    # maskT[p, t, b] = 1 if batch(128t + p) == b
    maskT_np = np.zeros((128, NT, BSH), dtype=np.float32)
    for t in range(NT):
        for p in range(128):
            bb = _batch_of_row(128 * t + p)
            maskT_np[p, t, bb] = 1.0
    # bsel[b, t, p]: one-hot selector, bsel.T @ att_h broadcasts per-row att_h
    bsel_np = np.ascontiguousarray(maskT_np.transpose(2, 1, 0))

    ident_c = nc.inline_tensor(ident_np, "c_ident")
    ones_c = nc.inline_tensor(ones_np, "c_ones")
    bsel_c = nc.inline_tensor(bsel_np.reshape(BSH, NT * 128), "c_bsel")
    maskT_c = nc.inline_tensor(maskT_np.reshape(128, NT * BSH), "c_maskT")

    with tile.TileContext(nc) as tc:
        import contextlib
        ctx = contextlib.ExitStack()
        with ctx:
            consts = ctx.enter_context(tc.tile_pool(name="consts", bufs=1))
            work = ctx.enter_context(tc.tile_pool(name="work", bufs=1))
            p_pool = ctx.enter_context(tc.tile_pool(name="p_pool", bufs=2))
            a_pool = ctx.enter_context(tc.tile_pool(name="a_pool", bufs=3))
            setup_sb_cm = tc.tile_pool(name="setup_sb", bufs=1)
            setup_sb = setup_sb_cm.__enter__()
            res_pool = ctx.enter_context(
                tc.tile_pool(name="respsum", bufs=1, space="PSUM"))
            setup_ps_cm = tc.tile_pool(name="setupps", bufs=2, space="PSUM")
            setup_ps = setup_ps_cm.__enter__()

            # ---- load constants and small inputs ----
            # W first on the ACT ring: it gates the att_h chain
            w_sb = setup_sb.tile([128, 4 * RNN], F32)
            w_view = w_d[:].rearrange("(c p) r -> p c r", p=128)
            for wc in range(4):
                nc.gpsimd.dma_start(
                    out=w_sb[:, wc * RNN:(wc + 1) * RNN],
                    in_=w_view[:, wc, :])
            h_sb = setup_sb.tile([BSH, RNN], F32)
            nc.gpsimd.dma_start(out=h_sb[:], in_=h_d[:])
            ident_sb = consts.tile([128, 128], F32R)
            nc.gpsimd.dma_start(out=ident_sb[:], in_=ident_c[:].bitcast(F32R))
            ident32_sb = consts.tile([128, 128], F32)
            nc.gpsimd.dma_start(out=ident32_sb[:], in_=ident_c[:])
            ones_sb = consts.tile([128, 128], F32R)
            nc.gpsimd.dma_start(out=ones_sb[:], in_=ones_c[:].bitcast(F32R))
            onesr_sb = consts.tile([128, 2], F32R)
            nc.gpsimd.dma_start(out=onesr_sb[:], in_=ones_c[:, 0:2].bitcast(F32R))
            bsel_sb = consts.tile([BSH, NT * 128], F32R)
            nc.scalar.dma_start(out=bsel_sb[:], in_=bsel_c[:].bitcast(F32R))
            bias_sb = setup_sb.tile([1, ATTH], F32)
            nc.gpsimd.dma_start(out=bias_sb[:], in_=bias_d[:])
            ones32_sb = consts.tile([1, 128], F32)
            nc.gpsimd.dma_start(out=ones32_sb[:], in_=ones_c[0:1, :])
            walpha_sb = setup_sb.tile([1, ATTH], F32R)
            nc.gpsimd.dma_start(out=walpha_sb[:], in_=walpha_d[:].bitcast(F32R))
            maskT_sb = consts.tile([128, NT * BSH], F32)
            nc.scalar.dma_start(out=maskT_sb[:], in_=maskT_c[:])

            # ---- transpose h -> hT [r, b] and W -> wT [r, a] via PE ----
            hT_sb = setup_sb.tile([128, 8 * BSH], F32)
            for rc in range(8):
                ps = setup_ps.tile([128, BSH], F32, tag="sps")
                nc.tensor.transpose(
                    ps[:], h_sb[:, rc * 128:(rc + 1) * 128],
                    ident32_sb[0:BSH, 0:BSH])
                nc.vector.tensor_copy(hT_sb[:, rc * BSH:(rc + 1) * BSH], ps[:])
            wT_sb = setup_sb.tile([128, 8 * ATTH], F32)
            for ac in range(4):
                for rq in range(2):  # rc quads: 4 transposes share a bank
                    ps = setup_ps.tile([128, 512], F32, tag="sps")
                    for j in range(4):
                        rc = rq * 4 + j
                        nc.tensor.transpose(
                            ps[:, j * 128:(j + 1) * 128],
                            w_sb[:, ac * RNN + rc * 128: ac * RNN + (rc + 1) * 128],
                            ident32_sb[:, :])
                    nc.vector.tensor_copy(
                        wT_sb[:].rearrange(
                            "p (r a) -> p r a", a=ATTH
                        )[:, rq * 4:(rq + 1) * 4, ac * 128:(ac + 1) * 128],
                        ps[:].rearrange("p (r a) -> p r a", a=128))

            # ---- att_h = h @ W.T + bias  -> [BSH, ATTH] (fp32 exact) ----
            ah_ps = setup_ps.tile([BSH, ATTH], F32, tag="sps")
            for rc in range(8):
                nc.tensor.matmul(
                    ah_ps[:],
                    lhsT=hT_sb[:, rc * BSH:(rc + 1) * BSH],
                    rhs=wT_sb[:, rc * ATTH:(rc + 1) * ATTH],
                    start=(rc == 0), stop=False)
            nc.tensor.matmul(
                ah_ps[:], lhsT=ones32_sb[0:1, 0:BSH], rhs=bias_sb[0:1, :],
                start=False, stop=True)
            atth32_sb = work.tile([BSH, ATTH], F32)
            nc.vector.tensor_copy(atth32_sb[:], ah_ps[:])
            atth_sb = work.tile([BSH, ATTH], F32R)
            nc.scalar.activation(atth_sb[:], atth32_sb[:], AF.Copy,
                                 bias=0.0, scale=1.0)

            # ---- broadcast w_alpha to all 128 partitions ----
            wb_ps = setup_ps.tile([128, ATTH], F32, tag="sps")
            nc.tensor.matmul(wb_ps[:], lhsT=ones_sb[0:1, 0:128],
                             rhs=walpha_sb[0:1, :], start=True, stop=True)
            wb_sb = work.tile([128, ATTH], F32)
            nc.vector.tensor_copy(wb_sb[:], wb_ps[:])
            setup_sb_cm.__exit__(None, None, None)
            setup_ps_cm.__exit__(None, None, None)

            zp_pool = ctx.enter_context(
                tc.tile_pool(name="zpsum", bufs=2, space="PSUM"))
            dot_pool = ctx.enter_context(tc.tile_pool(name="dot", bufs=3))
            prod_pool = ctx.enter_context(tc.tile_pool(name="prod", bufs=2))
            small_pool = ctx.enter_context(tc.tile_pool(name="small", bufs=4))

            # ---- persistent accumulators ----
            res_ps0 = res_pool.tile([BSH, 512], F32, tag="res0")
            res_ps1 = res_pool.tile([BSH, 512], F32, tag="res1")
            se_ps = res_pool.tile([BSH, 2], F32, tag="sumexp")

            p_view = p_d[:].rearrange("(t p) e -> p t e", p=128)
            a_view = att_d[:].rearrange("(t p) e -> p t e", p=128)

            p_tiles = {}
            a_tiles = {}
            GMAX = max(GROUPS)

            def load_p_group(g):
                lo = GSTART[g]
                hi = lo + GROUPS[g]
                t_ = p_pool.tile([128, GMAX * ATTH], F32R, tag="pg")
                nc.gpsimd.dma_start(
                    out=t_[:, 0:(hi - lo) * ATTH].rearrange(
                        "p (t e) -> p t e", e=ATTH),
                    in_=p_view[:, lo:hi, :].bitcast(F32R))
                for t in range(lo, hi):
                    p_tiles[t] = t_[:, (t - lo) * ATTH:(t - lo + 1) * ATTH]

            def load_a_group(g):
                lo = GSTART[g]
                hi = lo + GROUPS[g]
                t_ = a_pool.tile([128, GMAX * RNN], F32R, tag="ag")
                nc.sync.dma_start(
                    out=t_[:, 0:(hi - lo) * RNN].rearrange(
                        "p (t e) -> p t e", e=RNN),
                    in_=a_view[:, lo:hi, :].bitcast(F32R))
                for t in range(lo, hi):
                    a_tiles[t] = t_[:, (t - lo) * RNN:(t - lo + 1) * RNN]

            # ---- main loop over 49 row-tiles, grouped by DMA group ----
            for _rep, g in _rep_groups(repeats):
                lo = GSTART[g]
                hi = lo + GROUPS[g]
                load_p_group(g)
                load_a_group(g)

                scol_g = small_pool.tile([128, GROUPS[g]], F32, tag="scol")
                # pair tiles: one [128, 1024] PSUM (2 banks), one tanh call
                pairs = [(t, min(t + 1, hi - 1)) if t + 1 < hi else (t, None)
                         for t in range(lo, hi, 2)]
                for t0, t1 in pairs:
                    n_z = 1024 if t1 is not None else 512
                    z_ps = zp_pool.tile([128, 1024], F32, tag="z")
                    for i, t in enumerate((t0, t1)):
                        if t is None:
                            continue
                        zslice = z_ps[:, i * 512:(i + 1) * 512]
                        nc.tensor.matmul(
                            zslice, lhsT=ident_sb[:],
                            rhs=p_tiles[t], start=True, stop=False)
                        nc.tensor.matmul(
                            zslice,
                            lhsT=bsel_sb[:, t * 128:(t + 1) * 128],
                            rhs=atth_sb[:],
                            start=False, stop=True)

                    # dot = tanh(z) -> SBUF (one call per pair)
                    dot_sb = dot_pool.tile([128, 1024], F32, tag="dot")
                    nc.scalar.activation(dot_sb[:, 0:n_z], z_ps[:, 0:n_z],
                                         AF.Tanh)

                    for i, t in enumerate((t0, t1)):
                        if t is None:
                            continue
                        dslice = dot_sb[:, i * 512:(i + 1) * 512]
                        # prod = dot * w_alpha (VectorE)
                        prod_sb = prod_pool.tile([128, ATTH], F32, tag="prod")
                        nc.vector.tensor_tensor(
                            out=prod_sb[:], in0=dslice, in1=wb_sb[:],
                            op=ALU.mult)
                        # scores col = sum_a prod; alternate engines to
                        # balance ScalarE vs VectorE occupancy
                        if t % 2 == 0:
                            nc.scalar.activation(
                                dslice, prod_sb[:], AF.Copy, bias=0.0,
                                scale=1.0,
                                accum_out=scol_g[:, t - lo: t - lo + 1])
                        else:
                            nc.vector.tensor_reduce(
                                out=scol_g[:, t - lo: t - lo + 1],
                                in_=prod_sb[:], axis=mybir.AxisListType.X,
                                op=ALU.add)

                # e = exp(scores) for the whole group
                ecol_g = small_pool.tile([128, GROUPS[g]], F32, tag="ecol")
                nc.scalar.activation(ecol_g[:], scol_g[:], AF.Exp)

                for t in range(lo, hi):
                    # masked weight columns: lhsT[p, b] = e[p] * mask[p, b]
                    lhsT_t = small_pool.tile([128, BSH], F32R, tag="lhsT")
                    nc.vector.tensor_scalar(
                        out=lhsT_t[:], in0=maskT_sb[:, t * BSH:(t + 1) * BSH],
                        scalar1=ecol_g[:, t - lo: t - lo + 1], scalar2=None,
                        op0=ALU.mult)

                    # att_res += lhsT.T @ A ; sumexp += lhsT.T @ 1
                    nc.tensor.matmul(
                        res_ps0[:], lhsT=lhsT_t[:],
                        rhs=a_tiles[t][:, 0:512],
                        start=(t == 0), stop=(t == NT - 1))
                    nc.tensor.matmul(
                        res_ps1[:], lhsT=lhsT_t[:],
                        rhs=a_tiles[t][:, 512:1024],
                        start=(t == 0), stop=(t == NT - 1))
                    nc.tensor.matmul(
                        se_ps[:], lhsT=lhsT_t[:], rhs=onesr_sb[:],
                        start=(t == 0), stop=(t == NT - 1))

                # ---- finalize: out = att_res / sumexp (per repeat so no
                # repeat is dead code in benchmark builds) ----
                if g == len(GROUPS) - 1:
                    recip_sb = work.tile([BSH, 1], F32)
                    nc.vector.reciprocal(recip_sb[:], se_ps[:, 0:1])
                    out_sb = work.tile([BSH, RNN], F32)
                    nc.scalar.activation(out_sb[:, 0:512], res_ps0[:], AF.Copy,
                                         bias=0.0, scale=recip_sb[:, 0:1])
                    nc.sync.dma_start(out=out_d[:, 0:512],
                                      in_=out_sb[:, 0:512])
                    nc.scalar.activation(out_sb[:, 512:1024], res_ps1[:],
                                         AF.Copy,
                                         bias=0.0, scale=recip_sb[:, 0:1])
                    nc.sync.dma_start(out=out_d[:, 512:1024],
                                      in_=out_sb[:, 512:1024])

    nc.compile()
    return nc


def kernel(h, att_feats, p_att_feats, w_h2att, b_h2att, w_alpha, b_alpha):
    """Full-input entry point. b_alpha is dropped: softmax is shift-invariant."""
    if "nc" not in _cached:
        _cached["nc"] = build_nc()
    nc = _cached["nc"]

    h = np.asarray(h, dtype=np.float32)
    att_feats = np.asarray(att_feats, dtype=np.float32)
    p_att_feats = np.asarray(p_att_feats, dtype=np.float32)
    w_h2att = np.ascontiguousarray(np.asarray(w_h2att, dtype=np.float32))
    b_h2att = np.asarray(b_h2att, dtype=np.float32).reshape(1, ATTH)
    w_alpha = np.asarray(w_alpha, dtype=np.float32).reshape(1, ATTH)

    in_maps = []
    for c in range(NCORES):
        lo = c * BSH
        hi = lo + BSH
        in_maps.append({
            "h": np.ascontiguousarray(h[lo:hi]),
            "att": np.ascontiguousarray(
                att_feats[lo:hi].reshape(G, RNN)),
            "p_att": np.ascontiguousarray(
                p_att_feats[lo:hi].reshape(G, ATTH)),
            "w_h2att": w_h2att,
            "b_h2att": b_h2att,
            "w_alpha": w_alpha,
        })

    res = run_bass_kernel_spmd(nc, in_maps, list(range(NCORES)))
    out = np.concatenate([res.results[c]["out"] for c in range(NCORES)],
                         axis=0)
    return out.astype(np.float32)
